# revision 3
# baseline (speedup 1.0000x reference)
"""Trainium2 Bass kernel for nn_BottleneckBlock (half-channel causal attention).

Contract: kernel(**inputs) takes the FULL unsharded inputs (as produced by the
problem's setup_inputs) and returns the FULL output, distributing work across
8 NeuronCores data-parallel over the (b, h, w) positions.

Per-core math (144 positions, seq N=64, C=256, 8 heads x 32):
  qkv = x @ qkv_w.T + qkv_b ; causal softmax(q k^T / sqrt(32) + rpb) @ v ; proj.

v2 design notes:
  - QKV + proj matmuls run in fp8e4 DoubleRow perf mode (2 k-tiles of 128 per
    instruction, 0.5 cycles/row): weights are scaled x16 on the host so they
    sit in fp8 normal range; the 16*16=256 logit scale factor is folded into
    the exp() activation scale, and the 256x on the projection output is
    removed by a host-side epilogue together with the projection bias.
  - The k projection bias is dropped entirely: a per-query additive constant
    on the logits cancels in softmax.
  - S^T lives in one 4-bank PSUM tile -> exp is a single wide Scalar op and
    the exp(bias)+mask multiply is a single GpSimd op (GpSimd has no PSUM
    port, so it gets the only SBUF->SBUF stage). The AV output PSUM tile
    aliases the S banks (same pool tag) since S is dead after exp.
  - The softmax denominator comes out of the attention-value matmul via an
    appended all-ones column of V; V's bias is folded into the host epilogue
    bias (b_out' = proj_b + proj_w @ b_v).
  - PSUM->SBUF traffic is the co-bottleneck (DMA cannot touch PSUM): ops are
    consolidated and split Scalar/Vector by measured throughput.
"""

import os
import sys
from contextlib import ExitStack

import numpy as np

sys.path.insert(0, "/opt/trn_rl_repo")

import ml_dtypes

BF16 = ml_dtypes.bfloat16

# Problem shape (hardcoded per spec)
B, T, CH, HS, WS = 2, 64, 512, 24, 24
HALF = CH // 2          # 256
HEADS = 8
HD = 32
SCALE = HD ** -0.5
NCORES = 8
NPOS = B * HS * WS      # 1152 positions
POS_PER_CORE = NPOS // NCORES   # 144
TOK = POS_PER_CORE * T  # 9216 tokens per core
WSCALE = 16.0           # fp8 weight pre-scale (keeps weights in normal range)

_BUILD_CACHE = {}


def _f8(a):
    import concourse.mybir as mybir
    f8 = np.dtype(mybir.dt.np(mybir.dt.float8e4))
    return np.ascontiguousarray(np.clip(a, -240.0, 240.0).astype(f8))


def _bf(a):
    return np.ascontiguousarray(a.astype(BF16))


def _host_prep(x, rpb_table, qkv_w, qkv_b, proj_w, proj_b):
    """Build the 8 per-core input maps + keep what's needed for reassembly."""
    x = np.asarray(x, dtype=np.float32)
    qkv_w = np.asarray(qkv_w, dtype=np.float32)
    qkv_b = np.asarray(qkv_b, dtype=np.float32)
    proj_w = np.asarray(proj_w, dtype=np.float32)
    proj_b = np.asarray(proj_b, dtype=np.float32)
    rpb = np.asarray(rpb_table, dtype=np.float32)

    # ---- x transpose: (B,T,CH,H,W) attention half -> [c, B_*T] fp8 ----
    b_part = x[:, :, HALF:]                       # (B,T,256,H,W)
    xt = np.transpose(b_part, (2, 0, 3, 4, 1))    # (256, B, H, W, T)
    xt = np.ascontiguousarray(xt).reshape(2, 128, NPOS * T)
    xt_f8 = _f8(np.transpose(xt, (1, 0, 2)))      # [128, 2, NPOS*T]

    # ---- weights (x16, fp8, DoubleRow k-tile layout [128, 2, M]) ----
    wqk = np.empty((128, 2, 512), dtype=np.float32)
    for kc in range(2):
        wqk[:, kc, 0:256] = qkv_w[0:256].T[128 * kc: 128 * (kc + 1)] * WSCALE
        wqk[:, kc, 256:512] = qkv_w[256:512].T[128 * kc: 128 * (kc + 1)] * WSCALE
    # q bias only (k bias cancels in softmax); scaled x16 to match q' = 16*q
    bq = np.stack([qkv_b[0:128], qkv_b[128:256]], axis=1) * WSCALE  # [128, 2]

    wv = np.empty((128, 2, 256), dtype=np.float32)
    for kc in range(2):
        wv[:, kc, :] = qkv_w[512:768].T[128 * kc: 128 * (kc + 1)] * WSCALE

    wp = np.empty((128, 2, 256), dtype=np.float32)
    for w in range(2):
        wp[:, w, :] = proj_w.T[128 * w: 128 * (w + 1)] * WSCALE

    bv = qkv_b[512:768]
    bp_full = (proj_b + proj_w @ bv).astype(np.float32)   # host epilogue bias

    # ---- exp(bias)+causal-mask tiles, transposed & replicated ----
    pos = np.arange(T)
    rel = pos[None, :] - pos[:, None] + (T - 1)   # [i, j]
    bias = rpb[rel]                               # [i, j, heads]
    eb = np.exp(bias.transpose(2, 0, 1))          # [h, i, j]
    eb = eb * (pos[None, None, :] <= pos[None, :, None])  # zero j>i
    ebT = np.transpose(eb, (0, 2, 1))             # [h, j, i]
    # bank r = h%4; free slot = 4*(h//4) + (s//2) -> head h = r + 4*(slot//4)
    ebrep = np.empty((128, 4, 512), dtype=np.float32)
    for r in range(4):
        for q in range(2):
            for slot in range(8):
                h = r + 4 * (slot // 4)
                ebrep[64 * q: 64 * (q + 1), r, 64 * slot: 64 * (slot + 1)] = ebT[h]

    ident = np.eye(128, dtype=np.float32)
    wqk_f8, wv_f8, wp_f8 = _f8(wqk), _f8(wv), _f8(wp)
    ebrep_bf, ident_bf = _bf(ebrep), _bf(ident)
    bq = np.ascontiguousarray(bq.astype(np.float32))

    in_maps = []
    for m in range(NCORES):
        sl = xt_f8[:, :, m * TOK: (m + 1) * TOK]  # [128, 2, 9216]
        in_maps.append({
            "xT": np.ascontiguousarray(sl),
            "wqk": wqk_f8,
            "wv": wv_f8,
            "wp": wp_f8,
            "bq": bq,
            "ebrep": ebrep_bf,
            "ident": ident_bf,
        })
    return in_maps, bp_full


def _emit(nc, tc, d):
    """Emit the Tile kernel. d: dict of dram APs."""
    import concourse.bass as bass
    import concourse.mybir as mybir

    f32 = mybir.dt.float32
    bf16 = mybir.dt.bfloat16
    f8 = mybir.dt.float8e4
    AFT = mybir.ActivationFunctionType
    DR = mybir.MatmulPerfMode.DoubleRow
    EXP_SCALE = SCALE / (WSCALE * WSCALE)

    ctx = tc._emit_ctx  # ExitStack installed by caller

    consts = ctx.enter_context(tc.tile_pool(name="consts", bufs=1))
    persist = ctx.enter_context(tc.tile_pool(name="persist", bufs=1))
    sb = ctx.enter_context(tc.tile_pool(name="sb", bufs=4))
    sb_qk = ctx.enter_context(tc.tile_pool(name="sb_qk", bufs=4))
    sb_ep = ctx.enter_context(tc.tile_pool(name="sb_ep", bufs=4))
    # PSUM: ps_mm = 2 slots x 2 banks (qkv/transpose/proj rotation);
    #       ps_s  = 1 slot x 4 banks (S^T, then AV aliases it via same tag).
    ps_mm = ctx.enter_context(tc.tile_pool(name="ps_mm", bufs=2, space="PSUM"))
    ps_s = ctx.enter_context(tc.tile_pool(name="ps_s", bufs=1, space="PSUM"))

    # ---- load constants; ordered so round 0's deps land first ----
    wqk_sb = consts.tile([128, 2, 512], f8)
    nc.sync.dma_start(wqk_sb, d["wqk"])
    bq_sb = consts.tile([128, 2], f32)
    nc.sync.dma_start(bq_sb, d["bq"])
    xT_sb = persist.tile([128, 2, TOK], f8, name="xT_sb")
    nc.sync.dma_start(xT_sb[:, :, 0:512], d["xT"][:, :, 0:512])
    wv_sb = consts.tile([128, 2, 256], f8)
    nc.sync.dma_start(wv_sb, d["wv"])
    nc.sync.dma_start(xT_sb[:, :, 512:1024], d["xT"][:, :, 512:1024])
    ebrep_sb = consts.tile([128, 4, 512], bf16)
    nc.sync.dma_start(ebrep_sb, d["ebrep"])
    ident_sb = consts.tile([128, 128], bf16)
    nc.sync.dma_start(ident_sb, d["ident"])
    wp_sb = consts.tile([128, 2, 256], f8)
    nc.sync.dma_start(wp_sb, d["wp"])
    for c0 in range(1024, TOK, 2048):
        nc.sync.dma_start(
            xT_sb[:, :, c0: c0 + 2048], d["xT"][:, :, c0: c0 + 2048]
        )

    # ---- persistent V ([tok-chunk, head-slot 33 = 32 v + 1 one]) ----
    v_all = persist.tile([128, TOK // 128, 320], bf16, name="v_all")
    nc.vector.memset(
        v_all.rearrange("p u (h e) -> p u h e", h=8)[:, :, :, 32:33], 1.0
    )

    def bcast_free(ap, n):
        return bass.AP(ap.tensor, ap.offset, [list(x) for x in ap.ap] + [[0, n]])

    def qkv_phase(R):
        tok0 = 512 * R
        XS = xT_sb[:, :, tok0: tok0 + 512]
        # ================= QKV projections (fp8 DoubleRow) =================
        qps = ps_mm.tile([128, 2, 512], f32, name="qps", tag="mmps")
        for fc in range(2):
            nc.tensor.matmul(
                qps[:, fc, :], wqk_sb[:, :, 128 * fc: 128 * (fc + 1)], XS,
                start=True, stop=True, perf_mode=DR,
            )
        q2 = sb_qk.tile([128, 2, 512], bf16, name="q2", tag="q2")
        for fc in range(2):
            nc.vector.tensor_scalar_add(
                q2[:, fc, :], qps[:, fc, :], bq_sb[:, fc: fc + 1]
            )
        kps = ps_mm.tile([128, 2, 512], f32, name="kps", tag="mmps")
        for fc in range(2):
            nc.tensor.matmul(
                kps[:, fc, :], wqk_sb[:, :, 256 + 128 * fc: 256 + 128 * (fc + 1)],
                XS, start=True, stop=True, perf_mode=DR,
            )
        k2 = sb_qk.tile([128, 2, 512], bf16, name="k2", tag="k2")
        nc.vector.tensor_copy(k2, kps)
        vps = ps_mm.tile([128, 2, 512], f32, name="vps", tag="mmps")
        for vf in range(2):
            for tcix in range(2):
                t0 = tok0 + 256 * vf + 128 * tcix
                nc.tensor.matmul(
                    vps[:, vf, 256 * tcix: 256 * (tcix + 1)],
                    xT_sb[:, :, t0: t0 + 128],
                    wv_sb,
                    start=True, stop=True, perf_mode=DR,
                )
        src_v = vps.rearrange("p a (t h e) -> p (a t) h e", h=8, e=32)
        dst_v = v_all.rearrange("p u (h e) -> p u h e", h=8)[
            :, 4 * R: 4 * R + 4, :, 0:32]
        nc.scalar.activation(dst_v, src_v, AFT.Copy)

        return q2, k2

    def attn_phase(R, q2, k2):
        tok0 = 512 * R
        # ========== S^T = K Q^T: bank r=h%4 (one row-group per bank) ==========
        sps = ps_s.tile([128, 4, 512], f32, name="sps", tag="sbank")
        for s in range(8):
            c = s % 2
            for h in range(HEADS):
                r = h % 4
                slot = 4 * (h // 4) + s // 2
                hr = 32 * (h % 4)
                nc.tensor.matmul(
                    sps[64 * c: 64 * (c + 1), r, 64 * slot: 64 * (slot + 1)],
                    k2[hr: hr + 32, h // 4, 64 * s: 64 * (s + 1)],
                    q2[hr: hr + 32, h // 4, 64 * s: 64 * (s + 1)],
                    start=True, stop=True,
                    tile_position=(hr, 64 * c),
                )
        # one wide exp (Scalar), one wide exp(bias)*mask multiply (GpSimd)
        et = sb_ep.tile([128, 4, 512], bf16, name="et", tag="esb")
        nc.scalar.activation(et, sps, AFT.Exp, scale=EXP_SCALE)
        pt = sb_ep.tile([128, 4, 512], bf16, name="pt", tag="psb")
        nc.gpsimd.tensor_mul(pt, et, ebrep_sb)

        # ============ AV + denom + normalize (banks alias S^T's) ============
        avn = sb.tile([128, 1024], bf16, name="avn", tag="avn")
        avps = ps_s.tile([128, 4, 512], f32, name="avps", tag="sbank")
        for r in range(4):
            for s in range(8):
                a = s % 2
                for hh in range(2):
                    h = r + 4 * hh
                    b = 2 * (s // 2) + hh
                    slot = 4 * hh + s // 2
                    nc.tensor.matmul(
                        avps[64 * a: 64 * (a + 1), r, 64 * b: 64 * b + 33],
                        pt[64 * a: 64 * (a + 1), r, 64 * slot: 64 * (slot + 1)],
                        v_all[64 * a: 64 * (a + 1), 4 * R + s // 2, 40 * h: 40 * h + 33],
                        start=True, stop=True,
                        tile_position=(64 * a, 64 * a),
                    )
        rsb = sb_ep.tile([128, 4, 8], f32, name="rsb", tag="rsb")
        nc.vector.reciprocal(
            rsb, avps.rearrange("p r (b e) -> p r b e", b=8)[:, :, :, 32]
        )
        nc.vector.tensor_mul(
            avn.rearrange("p (b q e) -> p b q e", b=8, q=4),
            avps.rearrange("p r (b e) -> p b r e", b=8)[:, :, :, 0:32],
            bcast_free(rsb.rearrange("p r b -> p b r"), 32),
        )

        # ============ transpose via PE (x identity) + projection ============
        avt = sb.tile([128, 2, 4, 128], f8, name="avt", tag="avt")
        for half in range(2):
            tps = ps_mm.tile([128, 4, 128], f32, name="tps", tag="mmps")
            for g4 in range(4):
                g = 4 * half + g4
                nc.tensor.matmul(
                    tps[:, g4, :], avn[:, 128 * g: 128 * (g + 1)], ident_sb,
                    start=True, stop=True,
                )
            nc.scalar.activation(
                avt[:, :, 2 * half: 2 * half + 2, :],
                tps.rearrange("p (u w) f -> p w u f", w=2),
                AFT.Copy,
            )
        for ec in range(2):
            pps = ps_mm.tile([128, 512], f32, name="pps", tag="mmps")
            nc.tensor.matmul(
                pps, wp_sb[:, :, 128 * ec: 128 * (ec + 1)],
                avt.rearrange("p w u f -> p w (u f)"),
                start=True, stop=True, perf_mode=DR,
            )
            osb = sb.tile([128, 512], bf16, name="osb", tag=f"osb{ec}")
            nc.vector.tensor_copy(osb, pps)
            nc.sync.dma_start(
                d["outT"][128 * ec: 128 * (ec + 1), tok0: tok0 + 512], osb
            )

    NR = TOK // 512  # 18 rounds of 8 positions, software-pipelined by one round
    pend = None
    for R in range(NR):
        tiles = qkv_phase(R)
        if pend is not None:
            attn_phase(R - 1, *pend)
        pend = tiles
    attn_phase(NR - 1, *pend)


def build():
    """Build + compile the Bass program (cached)."""
    if "nc" in _BUILD_CACHE:
        return _BUILD_CACHE["nc"]
    import concourse.bass as bass
    import concourse.mybir as mybir
    import concourse.tile as tile
    from concourse import bacc

    f32 = mybir.dt.float32
    bf16 = mybir.dt.bfloat16
    f8 = mybir.dt.float8e4

    nc = bacc.Bacc("TRN2", target_bir_lowering=False, debug=False,
                   enable_asserts=False, num_devices=NCORES)
    d = {
        "xT": nc.dram_tensor("xT", [128, 2, TOK], f8, kind="ExternalInput").ap(),
        "wqk": nc.dram_tensor("wqk", [128, 2, 512], f8, kind="ExternalInput").ap(),
        "wv": nc.dram_tensor("wv", [128, 2, 256], f8, kind="ExternalInput").ap(),
        "wp": nc.dram_tensor("wp", [128, 2, 256], f8, kind="ExternalInput").ap(),
        "bq": nc.dram_tensor("bq", [128, 2], f32, kind="ExternalInput").ap(),
        "ebrep": nc.dram_tensor("ebrep", [128, 4, 512], bf16, kind="ExternalInput").ap(),
        "ident": nc.dram_tensor("ident", [128, 128], bf16, kind="ExternalInput").ap(),
        "outT": nc.dram_tensor("outT", [256, TOK], bf16, kind="ExternalOutput").ap(),
    }
    with tile.TileContext(nc) as tc:
        with ExitStack() as es:
            tc._emit_ctx = es
            _emit(nc, tc, d)
    nc.compile()
    _BUILD_CACHE["nc"] = nc
    return nc


def _install_ntff_hook():
    """Provide antenv.axon_hooks with a ctypes NTFF profiling hook if the
    image's antenv package lacks it (mirrors the agent-boot registration)."""
    import contextlib
    import ctypes
    import types

    try:
        from antenv.axon_hooks import get_axon_ntff_profile_hook  # noqa: F401
        return True
    except ImportError:
        pass
    so_path = "/opt/axon/libaxon_pjrt.so"
    if not os.path.exists(so_path):
        return False
    lib = ctypes.CDLL(so_path)
    if not hasattr(lib, "axon_start_nrt_profile"):
        return False
    lib.axon_start_nrt_profile.argtypes = [ctypes.POINTER(ctypes.c_int64), ctypes.c_size_t]
    lib.axon_start_nrt_profile.restype = ctypes.c_int64
    lib.axon_stop_nrt_profile.argtypes = [ctypes.c_char_p]
    lib.axon_stop_nrt_profile.restype = ctypes.c_int64

    @contextlib.contextmanager
    def _hook(output_dir, device_ids):
        import jax
        jax.devices()
        if device_ids:
            ids = (ctypes.c_int64 * len(device_ids))(*device_ids)
            rc = lib.axon_start_nrt_profile(ids, len(device_ids))
        else:
            rc = lib.axon_start_nrt_profile(None, 0)
        if rc != 0:
            raise RuntimeError(f"axon_start_nrt_profile rc={rc}")
        try:
            yield
        finally:
            n = lib.axon_stop_nrt_profile(str(output_dir).encode())
            print(f"profile: {n} file(s) written to {output_dir}", file=sys.stderr)

    import antenv
    mod = types.ModuleType("antenv.axon_hooks")
    _state = {"hook": _hook}
    mod.get_axon_ntff_profile_hook = lambda: _state["hook"]
    mod.set_axon_ntff_profile_hook = lambda h: _state.update(hook=h)
    sys.modules["antenv.axon_hooks"] = mod
    antenv.axon_hooks = mod
    return True


def kernel(x, rpb_table, qkv_w, qkv_b, proj_w, proj_b):
    in_maps, bp_full = _host_prep(x, rpb_table, qkv_w, qkv_b, proj_w, proj_b)
    nc = build()
    from concourse import bass_utils

    trace = bool(int(os.environ.get("BASS_KERNEL_TRACE", "0")))
    if trace:
        trace = _install_ntff_hook()
    try:
        res = bass_utils.run_bass_kernel_spmd(
            nc, in_maps, core_ids=list(range(NCORES)), trace=trace
        )
    except Exception:
        if not trace:
            raise
        import traceback
        traceback.print_exc()
        print("trace run failed; retrying without trace", file=sys.stderr)
        res = bass_utils.run_bass_kernel_spmd(
            nc, in_maps, core_ids=list(range(NCORES)), trace=False
        )
    if trace and res.exec_time_ns is not None:
        print(f"HW exec time: {res.exec_time_ns} ns")
        _BUILD_CACHE["exec_time_ns"] = res.exec_time_ns
        _BUILD_CACHE["profile_res"] = res

    x = np.asarray(x, dtype=np.float32)
    out = np.empty_like(x)
    out[:, :, :HALF] = x[:, :, :HALF]
    # outT per core: raw bf16 [256, 9216] = 256*(proj out before bias)
    inv = 1.0 / (WSCALE * WSCALE)
    attn = np.empty((HALF, NPOS, T), dtype=np.float32)
    for m in range(NCORES):
        o = res.results[m]["outT"].astype(np.float32) * inv + bp_full[:, None]
        attn[:, m * POS_PER_CORE: (m + 1) * POS_PER_CORE, :] = o.reshape(
            HALF, POS_PER_CORE, T
        )
    # (c, B, H, W, T) -> (B, T, c, H, W)
    attn = attn.reshape(HALF, B, HS, WS, T)
    out[:, :, HALF:] = np.transpose(attn, (1, 4, 0, 2, 3))
    return out


# revision 8
# speedup vs baseline: 1.3318x; 1.3318x over previous
"""Trainium2 Bass kernel for nn_BottleneckBlock (half-channel causal attention).

Contract: kernel(**inputs) takes the FULL unsharded inputs (as produced by the
problem's setup_inputs) and returns the FULL output, distributing work across
8 NeuronCores data-parallel over the (b, h, w) positions.

Per-core math (144 positions, seq N=64, C=256, 8 heads x 32):
  qkv = x @ qkv_w.T + qkv_b ; causal softmax(q k^T / sqrt(32) + rpb) @ v ; proj.

v2 design notes:
  - QKV + proj matmuls run in fp8e4 DoubleRow perf mode (2 k-tiles of 128 per
    instruction, 0.5 cycles/row): weights are scaled x16 on the host so they
    sit in fp8 normal range; the 16*16=256 logit scale factor is folded into
    the exp() activation scale, and the 256x on the projection output is
    removed by a host-side epilogue together with the projection bias.
  - The k projection bias is dropped entirely: a per-query additive constant
    on the logits cancels in softmax.
  - S^T lives in one 4-bank PSUM tile -> exp is a single wide Scalar op and
    the exp(bias)+mask multiply is a single GpSimd op (GpSimd has no PSUM
    port, so it gets the only SBUF->SBUF stage). The AV output PSUM tile
    aliases the S banks (same pool tag) since S is dead after exp.
  - The softmax denominator comes out of the attention-value matmul via an
    appended all-ones column of V; V's bias is folded into the host epilogue
    bias (b_out' = proj_b + proj_w @ b_v).
  - PSUM->SBUF traffic is the co-bottleneck (DMA cannot touch PSUM): ops are
    consolidated and split Scalar/Vector by measured throughput.
"""

import os
import sys
from contextlib import ExitStack

import numpy as np

sys.path.insert(0, "/opt/trn_rl_repo")

import ml_dtypes

BF16 = ml_dtypes.bfloat16

# Problem shape (hardcoded per spec)
B, T, CH, HS, WS = 2, 64, 512, 24, 24
HALF = CH // 2          # 256
HEADS = 8
HD = 32
SCALE = HD ** -0.5
NCORES = 8
NPOS = B * HS * WS      # 1152 positions
POS_PER_CORE = NPOS // NCORES   # 144
TOK = POS_PER_CORE * T  # 9216 tokens per core
WSCALE = 16.0           # fp8 weight pre-scale (keeps weights in normal range)

_BUILD_CACHE = {}


def _f8(a):
    import concourse.mybir as mybir
    f8 = np.dtype(mybir.dt.np(mybir.dt.float8e4))
    return np.ascontiguousarray(np.clip(a, -240.0, 240.0).astype(f8))


def _bf(a):
    return np.ascontiguousarray(a.astype(BF16))


def _host_prep(x, rpb_table, qkv_w, qkv_b, proj_w, proj_b):
    """Build the 8 per-core input maps + keep what's needed for reassembly."""
    x = np.asarray(x, dtype=np.float32)
    qkv_w = np.asarray(qkv_w, dtype=np.float32)
    qkv_b = np.asarray(qkv_b, dtype=np.float32)
    proj_w = np.asarray(proj_w, dtype=np.float32)
    proj_b = np.asarray(proj_b, dtype=np.float32)
    rpb = np.asarray(rpb_table, dtype=np.float32)

    # ---- x transpose: (B,T,CH,H,W) attention half -> fp8 [128, chunks, 2, 512]
    b_part = x[:, :, HALF:]                       # (B,T,256,H,W)
    xt = np.transpose(b_part, (2, 0, 3, 4, 1))    # (256, B, H, W, T)
    xt = np.ascontiguousarray(xt).reshape(2, 128, NPOS * T // 512, 512)
    xt_f8 = _f8(np.transpose(xt, (1, 2, 0, 3)))   # [128, chunks, 2, 512]

    # ---- weights (x16, fp8, DoubleRow k-tile layout [128, 2, M]) ----
    wqk = np.empty((128, 2, 512), dtype=np.float32)
    for kc in range(2):
        wqk[:, kc, 0:256] = qkv_w[0:256].T[128 * kc: 128 * (kc + 1)] * WSCALE
        wqk[:, kc, 256:512] = qkv_w[256:512].T[128 * kc: 128 * (kc + 1)] * WSCALE
    # q bias only (k bias cancels in softmax); scaled x16 to match q' = 16*q
    bq = np.stack([qkv_b[0:128], qkv_b[128:256]], axis=1) * WSCALE  # [128, 2]

    wv = np.empty((128, 2, 256), dtype=np.float32)
    for kc in range(2):
        wv[:, kc, :] = qkv_w[512:768].T[128 * kc: 128 * (kc + 1)] * WSCALE

    wp = np.empty((128, 2, 256), dtype=np.float32)
    for w in range(2):
        wp[:, w, :] = proj_w.T[128 * w: 128 * (w + 1)] * WSCALE

    bv = qkv_b[512:768]
    bp_full = (proj_b + proj_w @ bv).astype(np.float32)   # host epilogue bias

    # ---- exp(bias)+causal-mask tiles, transposed & replicated ----
    pos = np.arange(T)
    rel = pos[None, :] - pos[:, None] + (T - 1)   # [i, j]
    bias = rpb[rel]                               # [i, j, heads]
    eb = np.exp(bias.transpose(2, 0, 1))          # [h, i, j]
    eb = eb * (pos[None, None, :] <= pos[None, :, None])  # zero j>i
    ebT = np.transpose(eb, (0, 2, 1))             # [h, j, i]
    # bank r = h%4; free slot = 4*(h//4) + (s//2) -> head h = r + 4*(slot//4)
    ebrep = np.empty((128, 4, 512), dtype=np.float32)
    for r in range(4):
        for q in range(2):
            for slot in range(8):
                h = r + 4 * (slot // 4)
                ebrep[64 * q: 64 * (q + 1), r, 64 * slot: 64 * (slot + 1)] = ebT[h]

    ident = np.eye(128, dtype=np.float32)
    wqk_f8, wv_f8, wp_f8 = _f8(wqk), _f8(wv), _f8(wp)
    ebrep_bf, ident_bf = _bf(ebrep), _bf(ident)
    bq = np.ascontiguousarray(bq.astype(np.float32))

    NRC = TOK // 512
    in_maps = []
    for m in range(NCORES):
        sl = xt_f8[:, m * NRC: (m + 1) * NRC]     # [128, 18, 2, 512]
        in_maps.append({
            "xT": np.ascontiguousarray(sl),
            "wqk": wqk_f8,
            "wv": wv_f8,
            "wp": wp_f8,
            "bq": bq,
            "ebrep": ebrep_bf,
            "ident": ident_bf,
        })
    return in_maps, bp_full


def _emit(nc, tc, d):
    """Emit the Tile kernel. d: dict of dram APs."""
    import concourse.bass as bass
    import concourse.mybir as mybir

    f32 = mybir.dt.float32
    bf16 = mybir.dt.bfloat16
    f8 = mybir.dt.float8e4
    AFT = mybir.ActivationFunctionType
    DR = mybir.MatmulPerfMode.DoubleRow
    EXP_SCALE = SCALE / (WSCALE * WSCALE)

    ctx = tc._emit_ctx  # ExitStack installed by caller

    NR = TOK // 512

    consts = ctx.enter_context(tc.tile_pool(name="consts", bufs=1))
    persist = ctx.enter_context(tc.tile_pool(name="persist", bufs=1))
    sb = ctx.enter_context(tc.tile_pool(name="sb", bufs=4))
    sb_qk = ctx.enter_context(tc.tile_pool(name="sb_qk", bufs=4))
    sb_ep = ctx.enter_context(tc.tile_pool(name="sb_ep", bufs=4))
    # PSUM: ps_mm = 2 slots x 2 banks (qkv/transpose/proj rotation);
    #       ps_s  = 2 slots x 2 banks (S^T bank pairs; AV aliases via tags).
    ps_mm = ctx.enter_context(tc.tile_pool(name="ps_mm", bufs=2, space="PSUM"))
    ps_s = ctx.enter_context(tc.tile_pool(name="ps_s", bufs=1, space="PSUM"))

    # ---- load constants; ordered so round 0's deps land first ----
    wqk_sb = consts.tile([128, 2, 512], f8)
    nc.sync.dma_start(wqk_sb, d["wqk"])
    bq_sb = consts.tile([128, 2], f32)
    nc.sync.dma_start(bq_sb, d["bq"])
    xT_sb = persist.tile([128, NR, 2, 512], f8, name="xT_sb")
    nc.sync.dma_start(xT_sb[:, 0], d["xT"][:, 0])
    wv_sb = consts.tile([128, 2, 256], f8)
    nc.sync.dma_start(wv_sb, d["wv"])
    nc.sync.dma_start(xT_sb[:, 1], d["xT"][:, 1])
    ebrep_sb = consts.tile([128, 4, 512], bf16)
    nc.sync.dma_start(ebrep_sb, d["ebrep"])
    ident_sb = consts.tile([128, 128], bf16)
    nc.sync.dma_start(ident_sb, d["ident"])
    wp_sb = consts.tile([128, 2, 256], f8)
    nc.sync.dma_start(wp_sb, d["wp"])
    for c0 in range(2, NR, 2):
        nc.sync.dma_start(
            xT_sb[:, c0: c0 + 2], d["xT"][:, c0: c0 + 2]
        )

    # ---- persistent V ([tok-chunk, head-slot 33 = 32 v + 1 one]) ----
    v_all = persist.tile([128, TOK // 128, 320], bf16, name="v_all")
    nc.vector.memset(
        v_all.rearrange("p u (h e) -> p u h e", h=8)[:, :, :, 32:33], 1.0
    )

    def bcast_free(ap, n):
        return bass.AP(ap.tensor, ap.offset, [list(x) for x in ap.ap] + [[0, n]])

    def qkv_phase(R):
        XS = xT_sb[:, R]
        # ================= QKV projections (fp8 DoubleRow) =================
        qps = ps_mm.tile([128, 2, 512], f32, name="qps", tag="mmps")
        for fc in range(2):
            nc.tensor.matmul(
                qps[:, fc, :], wqk_sb[:, :, 128 * fc: 128 * (fc + 1)], XS,
                start=True, stop=True, perf_mode=DR,
            )
        q2 = sb_qk.tile([128, 2, 512], bf16, name="q2", tag="q2")
        for fc in range(2):
            nc.scalar.activation(
                q2[:, fc, :], qps[:, fc, :], AFT.Identity,
                bias=bq_sb[:, fc: fc + 1],
            )
        kps = ps_mm.tile([128, 2, 512], f32, name="kps", tag="mmps")
        for fc in range(2):
            nc.tensor.matmul(
                kps[:, fc, :], wqk_sb[:, :, 256 + 128 * fc: 256 + 128 * (fc + 1)],
                XS, start=True, stop=True, perf_mode=DR,
            )
        k2 = sb_qk.tile([128, 2, 512], bf16, name="k2", tag="k2")
        nc.vector.tensor_copy(k2, kps)
        vps = ps_mm.tile([128, 2, 512], f32, name="vps", tag="mmps")
        for vf in range(2):
            for tcix in range(2):
                t0 = 256 * vf + 128 * tcix
                nc.tensor.matmul(
                    vps[:, vf, 256 * tcix: 256 * (tcix + 1)],
                    XS[:, :, t0: t0 + 128],
                    wv_sb,
                    start=True, stop=True, perf_mode=DR,
                )
        src_v = vps.rearrange("p a (t h e) -> p (a t) h e", h=8, e=32)
        dst_v = v_all.rearrange("p u (h e) -> p u h e", h=8)[
            :, 4 * R: 4 * R + 4, :, 0:32]
        nc.scalar.activation(dst_v, src_v, AFT.Copy)

        return q2, k2

    def attn_phase(R, q2, k2):
        tok0 = 512 * R
        # ==== S^T = K Q^T by bank pair rp: banks r=2rp+{0,1}, r=h%4 ====
        sps = [ps_s.tile([128, 2, 512], f32, name=f"sps{rp}", tag=f"sb{rp}")
               for rp in range(2)]
        for rp in range(2):
            for s in range(8):
                c = s % 2
                for hq in range(4):
                    h = 2 * rp + (hq % 2) + 4 * (hq // 2)
                    r = h % 4
                    slot = 4 * (h // 4) + s // 2
                    hr = 32 * (h % 4)
                    nc.tensor.matmul(
                        sps[rp][64 * c: 64 * (c + 1), r - 2 * rp,
                                64 * slot: 64 * (slot + 1)],
                        k2[hr: hr + 32, h // 4, 64 * s: 64 * (s + 1)],
                        q2[hr: hr + 32, h // 4, 64 * s: 64 * (s + 1)],
                        start=True, stop=True,
                        tile_position=(hr, 64 * c),
                    )
        # exp per bank pair (Scalar); exp(bias)*mask multiply on V / GpSimd
        pt = []
        for rp in range(2):
            et = sb_ep.tile([128, 2, 512], bf16, name=f"et{rp}", tag=f"esb{rp}")
            nc.scalar.activation(et, sps[rp], AFT.Exp, scale=EXP_SCALE)
            ptt = sb_ep.tile([128, 2, 512], bf16, name=f"pt{rp}", tag=f"psb{rp}")
            eng = nc.vector if rp == 0 else nc.gpsimd
            eng.tensor_mul(ptt, et, ebrep_sb[:, 2 * rp: 2 * rp + 2, :])
            pt.append(ptt)

        # ============ AV + denom + normalize (banks alias S^T's) ============
        avn = sb.tile([128, 1024], bf16, name="avn", tag="avn")
        for rp in range(2):
            avps = ps_s.tile([128, 2, 512], f32, name=f"avps{rp}", tag=f"sb{rp}")
            for rr in range(2):
                r = 2 * rp + rr
                for s in range(8):
                    a = s % 2
                    for hh in range(2):
                        h = r + 4 * hh
                        b = 2 * (s // 2) + hh
                        slot = 4 * hh + s // 2
                        nc.tensor.matmul(
                            avps[64 * a: 64 * (a + 1), rr, 64 * b: 64 * b + 33],
                            pt[rp][64 * a: 64 * (a + 1), rr,
                                   64 * slot: 64 * (slot + 1)],
                            v_all[64 * a: 64 * (a + 1), 4 * R + s // 2,
                                  40 * h: 40 * h + 33],
                            start=True, stop=True,
                            tile_position=(64 * a, 64 * a),
                        )
            rsb = sb_ep.tile([128, 2, 8], f32, name=f"rsb{rp}", tag=f"rsb{rp}")
            nc.vector.reciprocal(
                rsb, avps.rearrange("p r (b e) -> p r b e", b=8)[:, :, :, 32]
            )
            nc.vector.tensor_mul(
                avn.rearrange("p (b q e) -> p b q e", b=8, q=4)[
                    :, :, 2 * rp: 2 * rp + 2, :],
                avps.rearrange("p r (b e) -> p b r e", b=8)[:, :, :, 0:32],
                bcast_free(rsb.rearrange("p r b -> p b r"), 32),
            )

        # ============ transpose via PE (x identity) + projection ============
        avt = sb.tile([128, 2, 4, 128], f8, name="avt", tag="avt")
        tps = ps_mm.tile([128, 2, 4, 128], f32, name="tps", tag="mmps")
        for half in range(2):
            for g4 in range(4):
                g = 4 * half + g4
                nc.tensor.matmul(
                    tps[:, half, g4, :], avn[:, 128 * g: 128 * (g + 1)],
                    ident_sb, start=True, stop=True,
                )
            nc.scalar.activation(
                avt[:, :, 2 * half: 2 * half + 2, :],
                tps[:, half].rearrange("p (u w) f -> p w u f", w=2),
                AFT.Copy,
            )
        pps = ps_mm.tile([128, 2, 512], f32, name="pps", tag="mmps")
        for ec in range(2):
            nc.tensor.matmul(
                pps[:, ec, :], wp_sb[:, :, 128 * ec: 128 * (ec + 1)],
                avt.rearrange("p w u f -> p w (u f)"),
                start=True, stop=True, perf_mode=DR,
            )
            osb = sb.tile([128, 512], bf16, name="osb", tag=f"osb{ec}")
            nc.vector.tensor_copy(osb, pps[:, ec, :])
            nc.sync.dma_start(
                d["outT"][128 * ec: 128 * (ec + 1), tok0: tok0 + 512], osb
            )

    # 18 rounds of 8 positions, software-pipelined by one round
    pend = None
    for R in range(NR):
        tiles = qkv_phase(R)
        if pend is not None:
            attn_phase(R - 1, *pend)
        pend = tiles
    attn_phase(NR - 1, *pend)


def build():
    """Build + compile the Bass program (cached)."""
    if "nc" in _BUILD_CACHE:
        return _BUILD_CACHE["nc"]
    import concourse.bass as bass
    import concourse.mybir as mybir
    import concourse.tile as tile
    from concourse import bacc

    f32 = mybir.dt.float32
    bf16 = mybir.dt.bfloat16
    f8 = mybir.dt.float8e4

    nc = bacc.Bacc("TRN2", target_bir_lowering=False, debug=False,
                   enable_asserts=False, num_devices=NCORES)
    d = {
        "xT": nc.dram_tensor("xT", [128, TOK // 512, 2, 512], f8, kind="ExternalInput").ap(),
        "wqk": nc.dram_tensor("wqk", [128, 2, 512], f8, kind="ExternalInput").ap(),
        "wv": nc.dram_tensor("wv", [128, 2, 256], f8, kind="ExternalInput").ap(),
        "wp": nc.dram_tensor("wp", [128, 2, 256], f8, kind="ExternalInput").ap(),
        "bq": nc.dram_tensor("bq", [128, 2], f32, kind="ExternalInput").ap(),
        "ebrep": nc.dram_tensor("ebrep", [128, 4, 512], bf16, kind="ExternalInput").ap(),
        "ident": nc.dram_tensor("ident", [128, 128], bf16, kind="ExternalInput").ap(),
        "outT": nc.dram_tensor("outT", [256, TOK], bf16, kind="ExternalOutput").ap(),
    }
    with tile.TileContext(nc) as tc:
        with ExitStack() as es:
            tc._emit_ctx = es
            _emit(nc, tc, d)
    nc.compile()
    _BUILD_CACHE["nc"] = nc
    return nc


def _install_ntff_hook():
    """Provide antenv.axon_hooks with a ctypes NTFF profiling hook if the
    image's antenv package lacks it (mirrors the agent-boot registration)."""
    import contextlib
    import ctypes
    import types

    try:
        from antenv.axon_hooks import get_axon_ntff_profile_hook  # noqa: F401
        return True
    except ImportError:
        pass
    so_path = "/opt/axon/libaxon_pjrt.so"
    if not os.path.exists(so_path):
        return False
    lib = ctypes.CDLL(so_path)
    if not hasattr(lib, "axon_start_nrt_profile"):
        return False
    lib.axon_start_nrt_profile.argtypes = [ctypes.POINTER(ctypes.c_int64), ctypes.c_size_t]
    lib.axon_start_nrt_profile.restype = ctypes.c_int64
    lib.axon_stop_nrt_profile.argtypes = [ctypes.c_char_p]
    lib.axon_stop_nrt_profile.restype = ctypes.c_int64

    @contextlib.contextmanager
    def _hook(output_dir, device_ids):
        import jax
        jax.devices()
        if device_ids:
            ids = (ctypes.c_int64 * len(device_ids))(*device_ids)
            rc = lib.axon_start_nrt_profile(ids, len(device_ids))
        else:
            rc = lib.axon_start_nrt_profile(None, 0)
        if rc != 0:
            raise RuntimeError(f"axon_start_nrt_profile rc={rc}")
        try:
            yield
        finally:
            n = lib.axon_stop_nrt_profile(str(output_dir).encode())
            print(f"profile: {n} file(s) written to {output_dir}", file=sys.stderr)

    import antenv
    mod = types.ModuleType("antenv.axon_hooks")
    _state = {"hook": _hook}
    mod.get_axon_ntff_profile_hook = lambda: _state["hook"]
    mod.set_axon_ntff_profile_hook = lambda h: _state.update(hook=h)
    sys.modules["antenv.axon_hooks"] = mod
    antenv.axon_hooks = mod
    return True


def kernel(x, rpb_table, qkv_w, qkv_b, proj_w, proj_b):
    in_maps, bp_full = _host_prep(x, rpb_table, qkv_w, qkv_b, proj_w, proj_b)
    nc = build()
    from concourse import bass_utils

    trace = bool(int(os.environ.get("BASS_KERNEL_TRACE", "0")))
    if trace:
        trace = _install_ntff_hook()
    try:
        res = bass_utils.run_bass_kernel_spmd(
            nc, in_maps, core_ids=list(range(NCORES)), trace=trace
        )
    except Exception:
        if not trace:
            raise
        import traceback
        traceback.print_exc()
        print("trace run failed; retrying without trace", file=sys.stderr)
        res = bass_utils.run_bass_kernel_spmd(
            nc, in_maps, core_ids=list(range(NCORES)), trace=False
        )
    if trace and res.exec_time_ns is not None:
        print(f"HW exec time: {res.exec_time_ns} ns")
        _BUILD_CACHE["exec_time_ns"] = res.exec_time_ns
        _BUILD_CACHE["profile_res"] = res

    x = np.asarray(x, dtype=np.float32)
    out = np.empty_like(x)
    out[:, :, :HALF] = x[:, :, :HALF]
    # outT per core: raw bf16 [256, 9216] = 256*(proj out before bias)
    inv = 1.0 / (WSCALE * WSCALE)
    attn = np.empty((HALF, NPOS, T), dtype=np.float32)
    for m in range(NCORES):
        o = res.results[m]["outT"].astype(np.float32) * inv + bp_full[:, None]
        attn[:, m * POS_PER_CORE: (m + 1) * POS_PER_CORE, :] = o.reshape(
            HALF, POS_PER_CORE, T
        )
    # (c, B, H, W, T) -> (B, T, c, H, W)
    attn = attn.reshape(HALF, B, HS, WS, T)
    out[:, :, HALF:] = np.transpose(attn, (1, 4, 0, 2, 3))
    return out
